# revision 6
# baseline (speedup 1.0000x reference)
"""MoE feed-forward (B=4,S=2048,D=1024,F=2048,E=8,top-2) on 8 trn2 NeuronCores.

Strategy (F-split tensor parallel — perfectly load balanced):
 - Host computes top-2 softmax routing and builds the expert-sorted column
   list (16384 token-expert pairs = 32 blocks of 512, exactly).
 - Core j owns F-slice [j*256, (j+1)*256) of every expert: W1/W3 column
   slices and the matching W2 row slice.  Every core processes ALL 16384
   columns — identical layout on all cores, so expert-segment boundaries
   are compile-time constants and the SPMD program is the same everywhere.
 - Per block (512 cols): h1 = W1_e^T x (2 f-tiles x 8 k), s = silu(h1),
   h3 = W3_e^T x, g = s*h3, y_partial = W2_e^T g (8 d-tiles x 2 kf).
   Expert boundaries inside a block just split the matmul free dim.
 - y partials (fp16) are summed over the 8 cores on the host, which also
   applies the top-2 combine weights in fp32 during the scatter-add.
"""

import numpy as np

import concourse.bass as bass
import concourse.tile as tile
from concourse import bacc, mybir
from concourse.bass_utils import run_bass_kernel_spmd

B, S, D, F, E, TOPK = 4, 2048, 1024, 2048, 8, 2
N_CORES = 8
KD = D // 128          # 8 contraction tiles for D
FS = F // N_CORES      # 256-wide F slice per core
NF = FS // 128         # 2 f-tiles per expert per core
KW = FS // 128         # 2 contraction tiles for the W2 pass

_nc_cache = {}


def _segments(counts, c0, c1):
    """Expert segments overlapping [c0, c1) in the expert-sorted layout."""
    segs = []
    lo = 0
    for e, c in enumerate(counts):
        hi = lo + c
        a, b = max(lo, c0), min(hi, c1)
        if a < b:
            segs.append((e, a - c0, b - c0))
        lo = hi
    return segs


def _build_nc(counts):
    """Per-core Bass program; counts = tokens per expert (sum = 32*512)."""
    f16 = mybir.dt.float16
    f32 = mybir.dt.float32
    C = int(sum(counts))
    assert C == 16384
    NBLK = C // 512
    NCHUNK = NBLK // 2          # x/y DMA granularity: 1024 columns

    nc = bacc.Bacc(None, target_bir_lowering=False, enable_partition_id=False,
                   monotonic_sem_count=0)
    xT = nc.dram_tensor("xT", [D, C], f16, kind="ExternalInput")
    W1 = nc.dram_tensor("W1", [D, F], f16, kind="ExternalInput")   # E*FS cols
    W3 = nc.dram_tensor("W3", [D, F], f16, kind="ExternalInput")
    W2 = nc.dram_tensor("W2", [F, D], f16, kind="ExternalInput")   # E*FS rows
    yT = nc.dram_tensor("yT", [D, C], f16, kind="ExternalOutput")

    def slab(t, c0, ncols):
        return t[:, c0:c0 + ncols].rearrange("(k p) n -> p k n", p=128)

    with tile.TileContext(nc) as tc:
        with (
            tc.tile_pool(name="wpool", bufs=1) as wpool,
            tc.tile_pool(name="xpool", bufs=3) as xpool,
            tc.tile_pool(name="gpool", bufs=2) as gpool,
            tc.tile_pool(name="spool", bufs=2) as spool,
            tc.tile_pool(name="ypool", bufs=2) as ypool,
            tc.tile_pool(name="ps1", bufs=2, space="PSUM") as ps1p,
            tc.tile_pool(name="ps3", bufs=2, space="PSUM") as ps3p,
            tc.tile_pool(name="psY", bufs=4, space="PSUM") as psYp,
        ):
            # Weight SBUF residency: W1/W3 as [128, KD, F], W2 as [128, E*KW, D]
            w1sb = []
            for k in range(KD):
                w1sb.append(wpool.tile([128, F], f16, tag=f"w1_{k}", name=f"w1_{k}"))
            w3sb = []
            for k in range(KD):
                w3sb.append(wpool.tile([128, F], f16, tag=f"w3_{k}", name=f"w3_{k}"))
            w2sb = wpool.tile([128, E * KW, D], f16, tag="w2")

            # First x chunk (2 blocks) then W1 k-tiles so pass 1 of block 0
            # can start as soon as x k0 + W1 k0 land; W3/W2 follow.
            x0 = xpool.tile([128, KD, 1024], f16, tag="x")
            nc.sync.dma_start(out=x0[:, 0:1, :], in_=slab(xT, 0, 1024)[:, 0:1, :])
            nc.sync.dma_start(out=w1sb[0], in_=W1[0:128, :])
            nc.sync.dma_start(out=x0[:, 1:KD, :], in_=slab(xT, 0, 1024)[:, 1:KD, :])
            for k in range(1, KD):
                nc.sync.dma_start(out=w1sb[k], in_=W1[k * 128:(k + 1) * 128, :])
            for k in range(KD):
                nc.sync.dma_start(out=w3sb[k], in_=W3[k * 128:(k + 1) * 128, :])
            nc.sync.dma_start(out=w2sb, in_=W2[:, :].rearrange("(k p) n -> p k n", p=128))

            xs = [None] * NCHUNK
            xs[0] = x0
            ys = [None] * NCHUNK

            for blk in range(NBLK):
                ch, half = blk // 2, blk % 2
                c0 = blk * 512
                if half == 0:
                    # prefetch the next chunk (triple-buffered pool keeps the
                    # current chunk + one in flight)
                    if ch + 1 < NCHUNK:
                        xs[ch + 1] = xpool.tile([128, KD, 1024], f16, tag="x",
                                                name=f"x{ch + 1}")
                        nc.sync.dma_start(out=xs[ch + 1],
                                          in_=slab(xT, (ch + 1) * 1024, 1024))
                    ys[ch] = ypool.tile([128, KD, 1024], f16, tag="y",
                                        name=f"y{ch}")
                xsb = xs[ch]
                ysb = ys[ch]
                hs = slice(half * 512, half * 512 + 512)
                segs = _segments(counts, c0, c0 + 512)

                # Pass 1: h1 = W1_e^T x per f-tile, silu
                sts = []
                for f in range(NF):
                    ps1 = ps1p.tile([128, 512], f32, tag="ps1")
                    for (e, lo, hi) in segs:
                        fs = slice(e * FS + f * 128, e * FS + (f + 1) * 128)
                        for k in range(KD):
                            nc.tensor.matmul(
                                ps1[:, lo:hi], lhsT=w1sb[k][:, fs],
                                rhs=xsb[:, k, half * 512 + lo:half * 512 + hi],
                                start=(k == 0), stop=(k == KD - 1),
                            )
                    s = spool.tile([128, 512], f16, tag=f"s{f}")
                    nc.scalar.activation(s, ps1,
                                         mybir.ActivationFunctionType.Silu)
                    sts.append(s)

                # Pass 2: h3 = W3_e^T x, g = s * h3
                gts = []
                for f in range(NF):
                    ps3 = ps3p.tile([128, 512], f32, tag="ps3")
                    for (e, lo, hi) in segs:
                        fs = slice(e * FS + f * 128, e * FS + (f + 1) * 128)
                        for k in range(KD):
                            nc.tensor.matmul(
                                ps3[:, lo:hi], lhsT=w3sb[k][:, fs],
                                rhs=xsb[:, k, half * 512 + lo:half * 512 + hi],
                                start=(k == 0), stop=(k == KD - 1),
                            )
                    g = gpool.tile([128, 512], f16, tag=f"g{f}")
                    nc.vector.tensor_mul(g, sts[f], ps3)
                    gts.append(g)

                # Pass 3: y_partial^T = W2_e^T g
                for dd in range(KD):
                    ds_ = slice(dd * 128, (dd + 1) * 128)
                    psy = psYp.tile([128, 512], f32, tag="psy")
                    for (e, lo, hi) in segs:
                        for kf in range(KW):
                            nc.tensor.matmul(
                                psy[:, lo:hi], lhsT=w2sb[:, e * KW + kf, ds_],
                                rhs=gts[kf][:, lo:hi],
                                start=(kf == 0), stop=(kf == KW - 1),
                            )
                    # alternate drain between scalar and vector engines
                    if dd % 2 == 0:
                        nc.scalar.copy(ysb[:, dd, hs], psy)
                    else:
                        nc.vector.tensor_copy(ysb[:, dd, hs], psy)

                if half == 1:
                    if blk == NBLK - 1:
                        # split the final writeback so the tail drain is short
                        for dd in range(KD):
                            nc.sync.dma_start(
                                out=slab(yT, ch * 1024, 1024)[:, dd:dd + 1, :],
                                in_=ysb[:, dd:dd + 1, :],
                            )
                    else:
                        nc.sync.dma_start(out=slab(yT, ch * 1024, 1024), in_=ysb)
    nc.finalize()
    return nc


def _route(x, Wg):
    """Top-2 softmax routing in float64 (matches the f32 reference selection)."""
    logits = x.astype(np.float64) @ Wg.astype(np.float64)
    logits -= logits.max(axis=-1, keepdims=True)
    g = np.exp(logits)
    g /= g.sum(axis=-1, keepdims=True)
    top_i = np.argpartition(-g, TOPK - 1, axis=-1)[:, :TOPK]      # [T, 2]
    tg = np.take_along_axis(g, top_i, axis=-1)
    tg = tg / tg.sum(axis=-1, keepdims=True)
    return top_i, tg


def run(inputs, trace=False, trace_cores=None):
    hidden_states = np.asarray(inputs["hidden_states"], dtype=np.float32)
    Wg = np.asarray(inputs["Wg"], dtype=np.float32)
    W1 = np.asarray(inputs["W1"], dtype=np.float32)
    W3 = np.asarray(inputs["W3"], dtype=np.float32)
    W2 = np.asarray(inputs["W2"], dtype=np.float32)

    x = hidden_states.reshape(-1, D)                              # [T, D]
    T = x.shape[0]
    top_i, tg = _route(x, Wg)

    idx = []
    wts = []
    for e in range(E):
        sel = top_i == e                                          # [T, 2]
        rows = np.where(sel.any(axis=-1))[0]
        idx.append(rows)
        wts.append(np.where(sel[rows, 0], tg[rows, 0], tg[rows, 1]))
    counts = tuple(len(r) for r in idx)
    assert sum(counts) == T * TOPK

    if counts not in _nc_cache:
        _nc_cache[counts] = _build_nc(counts)
    nc = _nc_cache[counts]

    order = np.concatenate(idx)                                   # [32*512]
    xTall = np.ascontiguousarray(x.T[:, order].astype(np.float16))  # [D, C]

    in_maps = []
    for j in range(N_CORES):
        cs = slice(j * FS, (j + 1) * FS)
        W1c = np.ascontiguousarray(
            np.transpose(W1[:, :, cs], (1, 0, 2)).reshape(D, E * FS)
        ).astype(np.float16)                                      # [D, E*FS]
        W3c = np.ascontiguousarray(
            np.transpose(W3[:, :, cs], (1, 0, 2)).reshape(D, E * FS)
        ).astype(np.float16)
        W2c = np.ascontiguousarray(
            W2[:, cs, :].reshape(E * FS, D)
        ).astype(np.float16)                                      # [E*FS, D]
        in_maps.append({"xT": xTall, "W1": W1c, "W3": W3c, "W2": W2c})

    kwargs = {}
    if trace:
        kwargs["trace"] = True
        kwargs["trace_cores"] = trace_cores or list(range(N_CORES))
    res = run_bass_kernel_spmd(nc, in_maps, list(range(N_CORES)), **kwargs)

    ysum = res.results[0]["yT"].astype(np.float32)
    for j in range(1, N_CORES):
        ysum += res.results[j]["yT"].astype(np.float32)           # [D, C]

    out = np.zeros((T, D), np.float32)
    lo = 0
    for e in range(E):
        c = counts[e]
        out[idx[e]] += wts[e][:, None].astype(np.float32) * ysum[:, lo:lo + c].T
        lo += c
    return out.reshape(B, S, D), res


def kernel(**inputs):
    out, _ = run(inputs, trace=False)
    return out
